# revision 9
# baseline (speedup 1.0000x reference)
"""GCNII (gnn_message_passing) on 8 Trainium2 NeuronCores.

Strategy (dst-sharded, feature-major on device):
  - 6250 dst nodes per core, grouped into 98 blocks of 64 (per-core balanced
    assignment so every block needs exactly M 128-edge tiles).
  - Per layer, a node-major f16 table of h~ = dinv * h lives in DRAM
    (rebuilt each layer by AllGather).  Per-edge rows are fetched with
    gpsimd.dma_gather (int16 indices; two overlapping APs into the table at
    row offsets 0 / NTAB-32768 cover all 50176 rows).
  - Segment-sum via PE: per tile, stationary = gathered [128e x 128f] f16,
    moving = static 0/1 scatter matrix S [128e x 64d] fp8 (SBUF-resident),
    PSUM-accumulated per block -> feature-major agg^T.
  - z1 = agg*dinv + (a/(1-a))h0 (BN scale-invariance folds (1-a));
    z2 = z1 + z1 @ (b/(1-b))W (same folding); BN stats via free-axis
    reduce + 1KB AllReduce; relu+affine on ACT; PE-transpose back to
    node-major -> AllGather.
"""

import os
import sys

sys.path.insert(0, "/opt/trn_rl_repo")

import numpy as np
import ml_dtypes

N = 50000
E = 800000
IN = 128
HID = 128
OUT = 64
L = 8
ALPHA = 0.1
THETA = 0.5
BN_EPS = 1e-5

NCORES = 8
BLK = 64
TILE = 128
PER_CORE = N // NCORES           # 6250
BLOCKS = (PER_CORE + BLK - 1) // BLK  # 98
SLOTS = BLOCKS * BLK             # 6272
NTAB = SLOTS * NCORES            # 50176
TAB_OFF = NTAB - 32768           # 17408
G = 7                            # blocks per gather call group
NG = BLOCKS // G                 # 14

last_exec_time_ns = None
last_profile = None
last_raw = None

_CHUNKS = []
_off = 0
while _off < SLOTS:
    _CHUNKS.append((_off, min(512, SLOTS - _off)))
    _off += 512


def _f16(x):
    return np.ascontiguousarray(np.asarray(x, dtype=np.float32)).astype(np.float16)


def _preprocess(edge_index):
    ei = np.asarray(edge_index).astype(np.int64)
    src = np.concatenate([ei[0], np.arange(N, dtype=np.int64)])
    dst = np.concatenate([ei[1], np.arange(N, dtype=np.int64)])
    deg = np.bincount(dst, minlength=N).astype(np.float32)
    dinv = np.where(deg > 0, 1.0 / np.sqrt(deg), 0.0).astype(np.float32)
    indeg = np.bincount(dst, minlength=N)  # includes self-loop

    # serpentine balanced assignment of local nodes -> (block, slot)
    pos_of_node = np.zeros(N, dtype=np.int64)
    max_load = 0
    for c in range(NCORES):
        lo = c * PER_CORE
        d = indeg[lo:lo + PER_CORE]
        order = np.argsort(-d, kind="stable")
        # 6272 slots = 64 passes x 98 blocks; pad virtual nodes at the end
        blk_seq = np.empty(SLOTS, dtype=np.int64)
        for p in range(BLK):
            row = np.arange(BLOCKS) if p % 2 == 0 else np.arange(BLOCKS)[::-1]
            blk_seq[p * BLOCKS:(p + 1) * BLOCKS] = row
        slot_seq = np.repeat(np.arange(BLK), BLOCKS)
        blk_of = np.empty(PER_CORE, dtype=np.int64)
        slot_of = np.empty(PER_CORE, dtype=np.int64)
        blk_of[order] = blk_seq[:PER_CORE]
        slot_of[order] = slot_seq[:PER_CORE]
        pos_of_node[lo:lo + PER_CORE] = blk_of * BLK + slot_of
        loads = np.bincount(blk_of, weights=d.astype(np.float64),
                            minlength=BLOCKS).astype(np.int64)
        max_load = max(max_load, int(loads.max()))
    M = (max_load + TILE - 1) // TILE

    table_row_of_node = (np.arange(N) // PER_CORE) * SLOTS + pos_of_node
    src_row = table_row_of_node[src]
    blk_id = (dst // PER_CORE) * BLOCKS + (pos_of_node[dst] // BLK)
    nblk = NCORES * BLOCKS
    e_low = np.bincount(blk_id[src_row < TAB_OFF], minlength=nblk)
    e_high = np.bincount(blk_id[src_row >= 32768], minlength=nblk)
    t0 = None
    for cand in range(1, M):
        if (e_low <= cand * TILE).all() and (e_high <= (M - cand) * TILE).all():
            t0 = cand
            break
    assert t0 is not None, "no feasible table split"
    t1 = M - t0
    cap0 = t0 * TILE

    idx_all = np.zeros((NCORES, BLOCKS * M * TILE), dtype=np.int16)
    slot_all = np.full((NCORES, BLOCKS * M * TILE), -1, dtype=np.int32)
    order = np.argsort(blk_id, kind="stable")
    s_sorted = src_row[order]
    d_sorted = (pos_of_node[dst] % BLK)[order]
    b_sorted = blk_id[order]
    starts = np.searchsorted(b_sorted, np.arange(nblk))
    ends = np.searchsorted(b_sorted, np.arange(nblk) + 1)
    for g in range(nblk):
        c, b = divmod(g, BLOCKS)
        rows = s_sorted[starts[g]:ends[g]]
        dslots = d_sorted[starts[g]:ends[g]]
        lo_m = rows < TAB_OFF
        hi_m = rows >= 32768
        mid = np.flatnonzero(~lo_m & ~hi_m)
        n_mid_to_t0 = min(cap0 - lo_m.sum(), len(mid))
        t0_sel = np.concatenate([np.flatnonzero(lo_m), mid[:n_mid_to_t0]])
        t1_sel = np.concatenate([mid[n_mid_to_t0:], np.flatnonzero(hi_m)])
        assert len(t0_sel) <= cap0 and len(t1_sel) <= t1 * TILE
        base = b * M * TILE
        idx_all[c, base:base + len(t0_sel)] = rows[t0_sel].astype(np.int16)
        slot_all[c, base:base + len(t0_sel)] = dslots[t0_sel]
        idx_all[c, base + cap0:base + cap0 + len(t1_sel)] = (
            rows[t1_sel] - TAB_OFF).astype(np.int16)
        slot_all[c, base + cap0:base + cap0 + len(t1_sel)] = dslots[t1_sel]

    # gather-call index arrays, wrapped for dma_gather
    n0 = G * cap0
    n1 = G * t1 * TILE
    flat = idx_all.reshape(NCORES, BLOCKS, M * TILE)
    idx0 = np.zeros((NCORES, 128, NG * (n0 // 16)), dtype=np.int16)
    idx1 = np.zeros((NCORES, 128, NG * (n1 // 16)), dtype=np.int16)

    def wrap(a):  # [n] -> [128, n//16]
        return np.tile(a.reshape(-1, 16).T, (8, 1))

    for c in range(NCORES):
        for g in range(NG):
            bs = slice(g * G, (g + 1) * G)
            a0 = flat[c, bs, :cap0].reshape(-1)
            a1 = flat[c, bs, cap0:].reshape(-1)
            idx0[c, :, g * (n0 // 16):(g + 1) * (n0 // 16)] = wrap(a0)
            idx1[c, :, g * (n1 // 16):(g + 1) * (n1 // 16)] = wrap(a1)

    # S matrices: [128 lanes, BLOCKS*M*64] fp8 (0/1)
    s_sb = np.zeros((NCORES, 128, BLOCKS * M * BLK), dtype=ml_dtypes.float8_e4m3)
    for c in range(NCORES):
        s3 = np.zeros((BLOCKS * M, TILE, BLK), dtype=np.float32)
        valid = np.flatnonzero(slot_all[c] >= 0)
        s3[valid // TILE, valid % TILE, slot_all[c][valid]] = 1.0
        s_sb[c] = s3.transpose(1, 0, 2).reshape(TILE, -1).astype(
            ml_dtypes.float8_e4m3)

    return dict(M=M, t0=t0, t1=t1, idx0=idx0, idx1=idx1, s_sb=s_sb,
                dinv=dinv, pos=pos_of_node)


def _build_program(M, t0, t1):
    import concourse.bacc as bacc
    import concourse.tile as tile
    import concourse.mybir as mybir
    from concourse import library_config
    import concourse.bass as bass

    f16 = mybir.dt.float16
    f32 = mybir.dt.float32
    fp8 = mybir.dt.float8e4
    i16 = mybir.dt.int16
    Alu = mybir.AluOpType
    Act = mybir.ActivationFunctionType

    cap0 = t0 * TILE
    n0 = G * cap0
    n1 = G * t1 * TILE
    NCH = len(_CHUNKS)

    nc = bacc.Bacc("TRN2", target_bir_lowering=False, debug=False,
                   num_devices=NCORES)

    # ---- I/O ----
    d_h0A = nc.dram_tensor("h0A", [128, SLOTS], f16, kind="ExternalInput")
    d_hrelu0 = nc.dram_tensor("hrelu0", [128, SLOTS], f16, kind="ExternalInput")
    d_shard0 = nc.dram_tensor("shard0", [SLOTS, 128], f16, kind="ExternalInput")
    d_dinvA = nc.dram_tensor("dinvA", [128, SLOTS], f16, kind="ExternalInput")
    d_wconv = nc.dram_tensor("wconv", [128, L * 128], f16, kind="ExternalInput")
    d_w1T = nc.dram_tensor("w1T", [128, OUT], f16, kind="ExternalInput")
    d_b1c = nc.dram_tensor("b1c", [OUT, 1], f32, kind="ExternalInput")
    d_gbn = nc.dram_tensor("gbn", [128, L], f32, kind="ExternalInput")
    d_bbn = nc.dram_tensor("bbn", [128, L], f32, kind="ExternalInput")
    d_ident = nc.dram_tensor("ident", [128, 128], f16, kind="ExternalInput")
    d_smat = nc.dram_tensor("smat", [128, BLOCKS * M * BLK], fp8,
                            kind="ExternalInput")
    d_idx0 = nc.dram_tensor("idx0", [128, NG * (n0 // 16)], i16,
                            kind="ExternalInput")
    d_idx1 = nc.dram_tensor("idx1", [128, NG * (n1 // 16)], i16,
                            kind="ExternalInput")
    d_outT = nc.dram_tensor("outT", [OUT, SLOTS], f32, kind="ExternalOutput")
    debug = os.environ.get("GCN_DEBUG") == "1"
    if debug:
        d_dz1 = nc.dram_tensor("dz1", [128, SLOTS], f16, kind="ExternalOutput")
        d_dz2 = nc.dram_tensor("dz2", [128, SLOTS], f16, kind="ExternalOutput")
        d_dh = nc.dram_tensor("dh", [128, SLOTS], f16, kind="ExternalOutput")
        d_dst = nc.dram_tensor("dst", [128, 2], f32, kind="ExternalOutput")
        d_dbn = nc.dram_tensor("dbn", [128, 2], f32, kind="ExternalOutput")
        d_dco = nc.dram_tensor("dco", [128, 6], f32, kind="ExternalOutput")
        d_dg0 = nc.dram_tensor("dg0", [128, G * t0, 128], f16, kind="ExternalOutput")

    with tile.TileContext(nc) as tc:
        with tc.tile_pool(name="dram", bufs=1, space="DRAM") as dram, \
             tc.tile_pool(name="pers", bufs=1) as pers, \
             tc.tile_pool(name="g0p", bufs=2) as g0p, \
             tc.tile_pool(name="g1p", bufs=2) as g1p, \
             tc.tile_pool(name="zp", bufs=1) as zp, \
             tc.tile_pool(name="smallp", bufs=2) as smallp, \
             tc.tile_pool(name="sqp", bufs=2) as sqp, \
             tc.tile_pool(name="pa", bufs=3, space="PSUM") as pa, \
             tc.tile_pool(name="pw", bufs=2, space="PSUM") as pw, \
             tc.tile_pool(name="pt", bufs=2, space="PSUM") as pt:

            nc.gpsimd.load_library(library_config.mlp)

            # ---- internal DRAM ----
            tables = [dram.tile([NTAB, 128], f16, addr_space="Shared",
                                name=f"table_{i}", tag=f"table_{i}")
                      for i in range(L)]
            shard_nm = dram.tile([SLOTS, 128], f16, name="shard_nm")
            bn_in = dram.tile([128, 2], f32, name="bn_in")
            bn_out = dram.tile([128, 2], f32, name="bn_out")

            # ---- persistent SBUF ----
            sb_dinvA = pers.tile([128, SLOTS], f16, name="sb_dinvA", tag="sb_dinvA")
            sb_h0A = pers.tile([128, SLOTS], f16, name="sb_h0A", tag="sb_h0A")
            sb_S = pers.tile([128, BLOCKS * M * BLK], fp8, name="sb_S", tag="sb_S")
            sb_idx0 = pers.tile([128, NG * (n0 // 16)], i16, name="sb_idx0", tag="sb_idx0")
            sb_idx1 = pers.tile([128, NG * (n1 // 16)], i16, name="sb_idx1", tag="sb_idx1")
            sb_wconv = pers.tile([128, L * 128], f16, name="sb_wconv", tag="sb_wconv")
            sb_w1T = pers.tile([128, OUT], f16, name="sb_w1T", tag="sb_w1T")
            sb_b1c = pers.tile([OUT, 1], f32, name="sb_b1c", tag="sb_b1c")
            sb_gbn = pers.tile([128, L], f32, name="sb_gbn", tag="sb_gbn")
            sb_bbn = pers.tile([128, L], f32, name="sb_bbn", tag="sb_bbn")
            sb_ident = pers.tile([128, 128], f16, name="sb_ident", tag="sb_ident")
            sb_z1 = pers.tile([128, SLOTS], f16, name="sb_z1", tag="sb_z1")
            sb_z2 = pers.tile([128, SLOTS], f16, name="sb_z2", tag="sb_z2")
            sb_h = pers.tile([128, SLOTS], f16, name="sb_h", tag="sb_h")
            sb_htnm = pers.tile([128, SLOTS // 128, 128], f16, name="sb_htnm", tag="sb_htnm")

            for dst_t, src_t in [(sb_dinvA, d_dinvA), (sb_h0A, d_h0A),
                                 (sb_S, d_smat), (sb_idx0, d_idx0),
                                 (sb_idx1, d_idx1), (sb_wconv, d_wconv),
                                 (sb_w1T, d_w1T), (sb_b1c, d_b1c),
                                 (sb_gbn, d_gbn), (sb_bbn, d_bbn),
                                 (sb_ident, d_ident), (sb_h, d_hrelu0)]:
                nc.sync.dma_start(out=dst_t[:], in_=src_t.ap())

            # initial table: copy host-built node-major shard, AllGather
            nc.sync.dma_start(out=shard_nm[:], in_=d_shard0.ap())
            nc.gpsimd.collective_compute(
                "AllGather", Alu.bypass,
                replica_groups=[list(range(NCORES))],
                ins=[shard_nm.opt()], outs=[tables[0].opt()])

            for li in range(L):
                table = tables[li]
                table_next = tables[li + 1] if li < L - 1 else None
                tab0 = table[0:32768, :]
                tab1 = table[TAB_OFF:NTAB, :]

                for g in range(NG):
                    g0 = g0p.tile([128, G * t0, 128], f16, name=f"g0_{li}_{g}",
                                  tag="g0")
                    g1 = g1p.tile([128, G * t1, 128], f16, name=f"g1_{li}_{g}",
                                  tag="g1")
                    nc.gpsimd.dma_gather(
                        g0[:], tab0,
                        sb_idx0[:, g * (n0 // 16):(g + 1) * (n0 // 16)],
                        n0, n0, 128, single_packet=False)
                    nc.gpsimd.dma_gather(
                        g1[:], tab1,
                        sb_idx1[:, g * (n1 // 16):(g + 1) * (n1 // 16)],
                        n1, n1, 128, single_packet=False)
                    if debug and li == 0 and g == 0:
                        nc.sync.dma_start(out=d_dg0.ap(), in_=g0[:])
                    for j in range(G):
                        b = g * G + j
                        agg = pa.tile([128, BLK], f32, name=f"agg_{li}_{b}",
                                      tag="agg")
                        for t in range(M):
                            if t < t0:
                                stat = g0[:, j * t0 + t, :]
                            else:
                                stat = g1[:, j * t1 + (t - t0), :]
                            mov = sb_S[:, (b * M + t) * BLK:(b * M + t + 1) * BLK]
                            nc.tensor.matmul(agg[:], lhsT=stat, rhs=mov,
                                             start=(t == 0), stop=(t == M - 1))
                        sl = slice(b * BLK, (b + 1) * BLK)
                        nc.vector.tensor_mul(sb_z1[:, sl], agg[:],
                                             sb_dinvA[:, sl])
                        nc.vector.tensor_add(sb_z1[:, sl], sb_z1[:, sl],
                                             sb_h0A[:, sl])

                # z2 = z1 + z1 @ W', BN partial sums
                sums = smallp.tile([128, NCH], f32, name=f"sums_{li}",
                                   tag="sums")
                sqs = smallp.tile([128, NCH], f32, name=f"sqs_{li}", tag="sqs")
                wsl = sb_wconv[:, li * 128:(li + 1) * 128]
                for ci, (off, w) in enumerate(_CHUNKS):
                    p = pw.tile([128, 512], f32, name=f"pw_{li}_{ci}", tag="pw")
                    nc.tensor.matmul(p[:, :w], lhsT=wsl,
                                     rhs=sb_z1[:, off:off + w],
                                     start=True, stop=True)
                    nc.vector.scalar_tensor_tensor(
                        out=sb_z2[:, off:off + w], in0=p[:, :w], scalar=1.0,
                        in1=sb_z1[:, off:off + w], op0=Alu.mult, op1=Alu.add,
                        accum_out=sums[:, ci:ci + 1])
                    sq = sqp.tile([128, 512], f16, name=f"sq_{li}_{ci}",
                                  tag="sq")
                    nc.scalar.activation(sq[:, :w], sb_z2[:, off:off + w],
                                         Act.Square,
                                         accum_out=sqs[:, ci:ci + 1])

                stats = smallp.tile([128, 2], f32, name=f"stats_{li}",
                                    tag="stats")
                nc.vector.tensor_reduce(stats[:, 0:1], sums[:],
                                        mybir.AxisListType.X, Alu.add)
                nc.vector.tensor_reduce(stats[:, 1:2], sqs[:],
                                        mybir.AxisListType.X, Alu.add)
                nc.sync.dma_start(out=bn_in[:], in_=stats[:])
                nc.gpsimd.collective_compute(
                    "AllReduce", Alu.add,
                    replica_groups=[list(range(NCORES))],
                    ins=[bn_in.opt()], outs=[bn_out.opt()])
                bn_sb = smallp.tile([128, 2], f32, name=f"bn_sb_{li}",
                                    tag="bn_sb")
                nc.sync.dma_start(out=bn_sb[:], in_=bn_out[:])

                coef = smallp.tile([128, 6], f32, name=f"coef_{li}", tag="coef")
                mean, ex2, var, rinv, s_f, b_f = (coef[:, k:k + 1]
                                                  for k in range(6))
                nc.vector.tensor_scalar_mul(mean, bn_sb[:, 0:1], 1.0 / N)
                nc.vector.tensor_scalar_mul(ex2, bn_sb[:, 1:2], 1.0 / N)
                nc.vector.tensor_mul(var, mean, mean)
                nc.vector.tensor_sub(var, ex2, var)
                nc.vector.tensor_scalar_add(var, var, BN_EPS)
                nc.vector.reciprocal(rinv, var)
                nc.scalar.sqrt(s_f, rinv)
                nc.vector.tensor_mul(s_f, s_f, sb_gbn[:, li:li + 1])
                nc.vector.tensor_mul(b_f, s_f, mean)
                nc.vector.tensor_sub(b_f, sb_bbn[:, li:li + 1], b_f)

                nc.scalar.activation(sb_h[:], sb_z2[:], Act.Relu,
                                     bias=b_f, scale=s_f)
                if debug and li == 0:
                    nc.sync.dma_start(out=d_dz1.ap(), in_=sb_z1[:])
                    nc.sync.dma_start(out=d_dz2.ap(), in_=sb_z2[:])
                    nc.sync.dma_start(out=d_dh.ap(), in_=sb_h[:])
                    nc.sync.dma_start(out=d_dst.ap(), in_=stats[:])
                    nc.sync.dma_start(out=d_dbn.ap(), in_=bn_sb[:])
                    nc.sync.dma_start(out=d_dco.ap(), in_=coef[:])

                if li < L - 1:
                    nc.vector.tensor_mul(sb_z1[:], sb_h[:], sb_dinvA[:])
                    for c_ in range(SLOTS // 128):
                        ptile = pt.tile([128, 128], f16,
                                        name=f"pt_{li}_{c_}", tag="pt")
                        nc.tensor.transpose(
                            ptile[:], sb_z1[:, c_ * 128:(c_ + 1) * 128],
                            sb_ident[:])
                        nc.vector.tensor_copy(sb_htnm[:, c_, :], ptile[:])
                    nm_ap = shard_nm.rearrange(
                        "(c p) f -> p c f", p=128)
                    nc.sync.dma_start(out=nm_ap, in_=sb_htnm[:])
                    nc.gpsimd.collective_compute(
                        "AllGather", Alu.bypass,
                        replica_groups=[list(range(NCORES))],
                        ins=[shard_nm.opt()], outs=[table_next.opt()])

            # output projection (kept transposed; host untransposes)
            for ci, (off, w) in enumerate(_CHUNKS):
                p = pw.tile([128, 512], f32, name=f"po_{ci}", tag="pw")
                nc.tensor.matmul(p[:OUT, :w], lhsT=sb_w1T[:],
                                 rhs=sb_h[:, off:off + w],
                                 start=True, stop=True)
                ot = sqp.tile([OUT, 512], f32, name=f"ot_{ci}", tag="ot")
                nc.scalar.activation(ot[:, :w], p[:OUT, :w],
                                     Act.Identity, bias=sb_b1c[:, 0:1])
                nc.sync.dma_start(out=d_outT.ap()[:, off:off + w],
                                  in_=ot[:, :w])

    nc.compile()
    return nc


def kernel(x, edge_index, lin0_w, lin0_b, lin1_w, lin1_b, conv_w, bn_gamma,
           bn_beta):
    global last_exec_time_ns, last_profile, last_raw
    from concourse.bass_utils import run_bass_kernel_spmd

    x = np.asarray(x, np.float32)
    lin0_w = np.asarray(lin0_w, np.float32)
    lin0_b = np.asarray(lin0_b, np.float32)
    lin1_w = np.asarray(lin1_w, np.float32)
    lin1_b = np.asarray(lin1_b, np.float32)
    conv_w = np.asarray(conv_w, np.float32)
    bn_gamma = np.asarray(bn_gamma, np.float32)
    bn_beta = np.asarray(bn_beta, np.float32)

    P = _preprocess(edge_index)
    M, t0, t1 = P["M"], P["t0"], P["t1"]
    dinv, pos = P["dinv"], P["pos"]

    # host-side input projection + per-core feature-major packing
    h0 = np.maximum(x @ lin0_w.T + lin0_b, 0.0).astype(np.float32)  # [N, HID]
    alpha_p = ALPHA / (1.0 - ALPHA)

    h0A_all = np.zeros((NCORES, 128, SLOTS), np.float16)
    hrelu0_all = np.zeros((NCORES, 128, SLOTS), np.float16)
    shard0_all = np.zeros((NCORES, SLOTS, 128), np.float16)
    dinvA_all = np.zeros((NCORES, 128, SLOTS), np.float16)
    for c in range(NCORES):
        lo = c * PER_CORE
        p_ = pos[lo:lo + PER_CORE]
        hT = np.zeros((128, SLOTS), np.float32)
        hT[:, p_] = h0[lo:lo + PER_CORE].T
        dcol = np.zeros(SLOTS, np.float32)
        dcol[p_] = dinv[lo:lo + PER_CORE]
        hrelu0_all[c] = _f16(hT)
        h0A_all[c] = _f16(hT * alpha_p)
        dinvA_all[c] = _f16(np.tile(dcol, (128, 1)))
        ht = hrelu0_all[c].astype(np.float32) * dinvA_all[c].astype(np.float32)
        shard0_all[c] = _f16(ht).T

    betas = [float(np.log(THETA / (i + 1) + 1.0)) for i in range(L)]
    wconv = np.concatenate(
        [_f16(conv_w[i] * (betas[i] / (1.0 - betas[i]))) for i in range(L)],
        axis=1)  # [128, L*128], stationary [k=f_in, m=f_out]
    w1T = _f16(lin1_w.T)                       # [128, OUT]
    b1c = lin1_b.reshape(OUT, 1).astype(np.float32)
    gbn = np.ascontiguousarray(bn_gamma.T).astype(np.float32)   # [128, L]
    bbn = np.ascontiguousarray(bn_beta.T).astype(np.float32)
    ident = np.eye(128, dtype=np.float16)

    nc = _build_program(M, t0, t1)

    in_maps = []
    for c in range(NCORES):
        in_maps.append({
            "h0A": h0A_all[c], "hrelu0": hrelu0_all[c],
            "shard0": shard0_all[c], "dinvA": dinvA_all[c],
            "wconv": wconv, "w1T": w1T, "b1c": b1c,
            "gbn": gbn, "bbn": bbn, "ident": ident,
            "smat": P["s_sb"][c], "idx0": P["idx0"][c], "idx1": P["idx1"][c],
        })

    trace = os.environ.get("GCN_TRACE") == "1"
    if trace:
        # antenv may resolve from a read-only site lacking axon_hooks;
        # inject our shim as the submodule.
        import importlib.util as _ilu
        import antenv as _antenv
        if not hasattr(_antenv, "axon_hooks"):
            _spec = _ilu.spec_from_file_location(
                "antenv.axon_hooks", "/opt/trn_rl_repo/antenv/axon_hooks.py")
            _mod = _ilu.module_from_spec(_spec)
            _spec.loader.exec_module(_mod)
            sys.modules["antenv.axon_hooks"] = _mod
            _antenv.axon_hooks = _mod
    res = run_bass_kernel_spmd(nc, in_maps, list(range(NCORES)), trace=trace)
    last_exec_time_ns = res.exec_time_ns
    last_profile = res.profile_json
    last_raw = (res.results, P)

    out = np.zeros((N, OUT), np.float32)
    for c in range(NCORES):
        lo = c * PER_CORE
        out[lo:lo + PER_CORE] = res.results[c]["outT"][:, pos[lo:lo + PER_CORE]].T
    return out
